# revision 7
# baseline (speedup 1.0000x reference)
"""Trainium2 Bass kernel for the dual-attention LSTM encoder.

Math notes (verified against the reference):
  - The attention logits are e[b,k] = h.Wa_h + c.Wa_c + feats[b,k,:].Wa_f + ba.
    The h/c terms are constant across k, so they cancel in the softmax over k:
    attention weights depend only on the (fixed) per-feature time series. They
    are therefore constant across all time steps and can be precomputed once.
  - The model then reduces to: a1 = softmax_k(x_y . waf), a2 = softmax_k(mm . waf),
    xt1 = a1*x_y, xt2 = a2*mm, followed by two independent LSTMs over T-1 steps.
  - X_tilde = xt1, X_encoded[:, t] = [h1_t, h2_t].

Device strategy (per core, batch-sharded 2048/8 = 256):
  - Phase 1: build interleaved x_y / mm tiles, softmax over features, scale,
    DMA X_tilde out, transpose scaled inputs to (t*k, batch) layout via TensorE.
  - Phase 2: 127 recurrent steps; gates on partitions, batch on free dim.
    Matmuls in bf16 (error ~3e-3), psum f32, cell state f32.
    Gate-block column order is [i, f, o, g] (host pre-permutes weights) so one
    sigmoid activation covers i|f|o contiguously in PSUM.
    LSTM2's bias rides in the padded 16th input lane (constant 1.0).
"""

import numpy as np
import ml_dtypes

from concourse import bacc, mybir
from concourse.tile import TileContext
from concourse.bass_utils import run_bass_kernel_spmd
from concourse.masks import make_identity

BF16 = ml_dtypes.bfloat16
F32 = mybir.dt.float32
BF = mybir.dt.bfloat16
ALU = mybir.AluOpType
ACTF = mybir.ActivationFunctionType

B, T, H = 2048, 127, 128          # T = T_full - 1 = scan length
K = 16                            # padded feature count (K1=16, K2=15->16)
NCORES = 8
BC = B // NCORES                  # 256 batch per core
NB = T * K                        # 2032 interleaved cols
NBP = 2048                        # padded to 16 transpose blocks of 128
NJ = NBP // 128                   # 16 transpose blocks (8 timesteps each)

_cache = {}


def _build(l1_bias: bool):
    nc = bacc.Bacc("TRN2", target_bir_lowering=False, debug=False,
                   num_devices=NCORES)

    Xs = nc.declare_dram_parameter("Xs", [BC, T * 15], F32, isOutput=False)
    ys = nc.declare_dram_parameter("ys", [BC, T], F32, isOutput=False)
    wafrep = nc.declare_dram_parameter("wafrep", [128, T], F32, isOutput=False)
    WhhT1 = nc.declare_dram_parameter("WhhT1", [128, 512], BF, isOutput=False)
    WhhT2 = nc.declare_dram_parameter("WhhT2", [128, 512], BF, isOutput=False)
    # Wih.T embedded at partition rows 16r..16r+16 for r = t%8, zeros elsewhere,
    # so the x-part matmul can contract the full 128-partition transpose block.
    WihS1 = nc.declare_dram_parameter("WihS1", [128, 8 * 512], BF, isOutput=False)
    WihS2 = nc.declare_dram_parameter("WihS2", [128, 8 * 512], BF, isOutput=False)
    if l1_bias:
        b1p = nc.declare_dram_parameter("b1p", [128, 4], F32, isOutput=False)
    Xt_out = nc.declare_dram_parameter("Xt_out", [2, 128, NB], F32, isOutput=True)
    Xe_out = nc.declare_dram_parameter("Xe_out", [T, 128, 512], BF, isOutput=True)

    with TileContext(nc) as tc:
        import contextlib
        with contextlib.ExitStack() as ctx:
            consts = ctx.enter_context(tc.tile_pool(name="consts", bufs=1))
            ident = consts.tile([128, 128], F32)
            make_identity(nc, ident[:])

            whh1 = consts.tile([128, 512], BF, tag="w1")
            whh2 = consts.tile([128, 512], BF, tag="w2")
            wih1 = consts.tile([128, 8 * 512], BF, tag="w3")
            wih2 = consts.tile([128, 8 * 512], BF, tag="w4")
            nc.sync.dma_start(out=whh1[:], in_=WhhT1[:])
            nc.sync.dma_start(out=whh2[:], in_=WhhT2[:])
            nc.sync.dma_start(out=wih1[:], in_=WihS1[:])
            nc.sync.dma_start(out=wih2[:], in_=WihS2[:])
            wafsb = consts.tile([128, T], F32, tag="waf")
            nc.sync.dma_start(out=wafsb[:], in_=wafrep[:])
            if l1_bias:
                b1sb = consts.tile([128, 4], F32, tag="b1")
                nc.sync.dma_start(out=b1sb[:], in_=b1p[:])

            # persistent state + transposed-input storage
            state = ctx.enter_context(tc.tile_pool(name="state", bufs=1))
            xyT = state.tile([128, NJ * 256], BF, tag="xyT")   # (tk, j, b)
            mmT = state.tile([128, NJ * 256], BF, tag="mmT")
            Ct = state.tile([128, 512], F32, tag="C")          # [c1|c2]
            nc.vector.memset(Ct[:], 0.0)

            # ---------------- Phase 1: attention precompute -------------
            with contextlib.ExitStack() as p1:
                pool = p1.enter_context(tc.tile_pool(name="p1", bufs=2))
                psum = p1.enter_context(
                    tc.tile_pool(name="p1ps", bufs=2, space="PSUM"))

                for half in range(2):
                    bsl = slice(half * 128, (half + 1) * 128)
                    xsb = pool.tile([128, T * 15], F32, tag="xsb")
                    ysb = pool.tile([128, T], F32, tag="ysb")
                    nc.sync.dma_start(out=xsb[:], in_=Xs[bsl])
                    nc.sync.dma_start(out=ysb[:], in_=ys[bsl])
                    xs3 = xsb[:].rearrange("p (t k) -> p t k", k=15)

                    XY = pool.tile([128, NBP], F32, tag="XY")
                    MM = pool.tile([128, NBP], F32, tag="MM")
                    nc.vector.memset(XY[:, NB:], 0.0)
                    nc.vector.memset(MM[:, NB:], 0.0)
                    xy3 = XY[:, :NB].rearrange("p (t k) -> p t k", k=K)
                    mm3 = MM[:, :NB].rearrange("p (t k) -> p t k", k=K)

                    # x_y = [X | y], mm = [X*y | 1.0 (bias lane)]
                    nc.scalar.copy(out=xy3[:, :, 0:15], in_=xs3)
                    nc.scalar.copy(out=xy3[:, :, 15:16], in_=ysb[:, :, None])
                    nc.vector.tensor_tensor(
                        out=mm3[:, :, 0:15], in0=xs3,
                        in1=ysb[:, :, None].broadcast_to((128, T, 15)),
                        op=ALU.mult)
                    nc.vector.memset(mm3[:, :, 15:16], 1.0)

                    # s[b,k] = sum_t feats[b,t,k] * waf[t]
                    for nm, src, kk in (("xy", XY, K), ("mm", MM, 15)):
                        s3 = src[:, :NB].rearrange("p (t k) -> p k t", k=K)
                        tmp = pool.tile([128, K * T], F32, tag="tmp")
                        t3 = tmp[:].rearrange("p (k t) -> p k t", t=T)
                        nc.vector.tensor_tensor(
                            out=t3[:, 0:kk, :], in0=s3[:, 0:kk, :],
                            in1=wafsb[:, None, :].broadcast_to((128, kk, T)),
                            op=ALU.mult)
                        sred = pool.tile([128, K], F32, tag="sred")
                        nc.vector.tensor_reduce(
                            out=sred[:, 0:kk], in_=t3[:, 0:kk, :],
                            axis=mybir.AxisListType.X, op=ALU.add)
                        nmax = pool.tile([128, 1], F32, tag="nmax")
                        nc.vector.tensor_reduce(
                            out=nmax[:], in_=sred[:, 0:kk],
                            axis=mybir.AxisListType.X, op=ALU.max, negate=True)
                        ex = pool.tile([128, K], F32, tag="ex")
                        nc.scalar.activation(
                            out=ex[:, 0:kk], in_=sred[:, 0:kk], func=ACTF.Exp,
                            bias=nmax[:])
                        den = pool.tile([128, 1], F32, tag="den")
                        nc.vector.tensor_reduce(
                            out=den[:], in_=ex[:, 0:kk],
                            axis=mybir.AxisListType.X, op=ALU.add)
                        rec = pool.tile([128, 1], F32, tag="rec")
                        nc.vector.reciprocal(out=rec[:], in_=den[:])
                        att = pool.tile([128, K], F32, tag="att")
                        nc.vector.tensor_scalar(
                            out=att[:, 0:kk], in0=ex[:, 0:kk], scalar1=rec[:],
                            scalar2=None, op0=ALU.mult)
                        # scale features in place (mm: skip the bias lane)
                        tgt = xy3 if nm == "xy" else mm3
                        nc.vector.tensor_tensor(
                            out=tgt[:, :, 0:kk], in0=tgt[:, :, 0:kk],
                            in1=att[:, None, 0:kk].broadcast_to((128, T, kk)),
                            op=ALU.mult)

                    # X_tilde out (scaled x_y, original layout)
                    nc.sync.dma_start(out=Xt_out[half], in_=XY[:, :NB])

                    # transposes: (b, t*k) -> (t*k, b) in blocks of 128 cols
                    for nm, src, dstT in (("xy", XY, xyT), ("mm", MM, mmT)):
                        dst3 = dstT[:].rearrange("p (j b) -> p j b", b=256)
                        for jg in range(NJ // 4):
                            ps = psum.tile([128, 512], F32, tag="tps")
                            for q in range(4):
                                j = jg * 4 + q
                                nc.tensor.transpose(
                                    ps[:, q * 128:(q + 1) * 128],
                                    src[:, j * 128:(j + 1) * 128],
                                    ident[:])
                            eng = nc.scalar if nm == "xy" else nc.vector
                            if nm == "xy":
                                nc.scalar.copy(
                                    out=dst3[:, jg * 4:jg * 4 + 4, bsl],
                                    in_=ps[:].rearrange("p (q b) -> p q b", b=128))
                            else:
                                nc.vector.tensor_copy(
                                    out=dst3[:, jg * 4:jg * 4 + 4, bsl],
                                    in_=ps[:].rearrange("p (q b) -> p q b", b=128))

            # ---------------- Phase 2: recurrence ------------------------
            xyT3 = xyT[:].rearrange("p (j b) -> p j b", b=256)
            mmT3 = mmT[:].rearrange("p (j b) -> p j b", b=256)

            loop = ctx.enter_context(tc.tile_pool(name="loop", bufs=2))
            ps2 = ctx.enter_context(tc.tile_pool(name="ps2", bufs=2, space="PSUM"))

            hprev = None
            for t in range(T):
                j, r = divmod(t, 8)
                P1 = ps2.tile([128, 1024], F32, tag="P1")
                P2 = ps2.tile([128, 1024], F32, tag="P2")
                for (P, wih, whh, xT, hs) in (
                        (P1, wih1, whh1, xyT3, slice(0, 256)),
                        (P2, wih2, whh2, mmT3, slice(256, 512))):
                    wih3 = wih[:].rearrange("p (r g) -> p r g", g=512)
                    rhs_x = xT[:, j, :]
                    for q in range(4):
                        osl = P[:, q * 256:(q + 1) * 256]
                        nc.tensor.matmul(
                            osl, wih3[:, r, q * 128:(q + 1) * 128], rhs_x,
                            start=True, stop=(t == 0))
                        if t > 0:
                            nc.tensor.matmul(
                                osl, whh[:, q * 128:(q + 1) * 128],
                                hprev[:, hs], start=False, stop=True)
                if l1_bias:
                    for q in range(4):
                        nc.vector.tensor_scalar(
                            out=P1[:, q * 256:(q + 1) * 256],
                            in0=P1[:, q * 256:(q + 1) * 256],
                            scalar1=b1sb[:, q:q + 1], scalar2=None, op0=ALU.add)

                SIG = loop.tile([128, 1536], BF, tag="SIG")
                TG = loop.tile([128, 512], BF, tag="TG")
                TC = loop.tile([128, 512], BF, tag="TC")
                Mt = loop.tile([128, 512], BF, tag="Mt")
                Ht = loop.tile([128, 512], BF, tag="Ht")
                for li, P in ((0, P1), (1, P2)):
                    o0 = li * 768
                    nc.scalar.activation(
                        out=SIG[:, o0:o0 + 768], in_=P[:, 0:768], func=ACTF.Sigmoid)
                    nc.scalar.activation(
                        out=TG[:, li * 256:li * 256 + 256], in_=P[:, 768:1024],
                        func=ACTF.Tanh)
                for li in range(2):
                    o0, cs = li * 768, slice(li * 256, li * 256 + 256)
                    nc.vector.tensor_tensor(
                        out=Mt[:, cs], in0=SIG[:, o0:o0 + 256], in1=TG[:, cs],
                        op=ALU.mult)
                    nc.vector.tensor_tensor(
                        out=Ct[:, cs], in0=SIG[:, o0 + 256:o0 + 512],
                        in1=Ct[:, cs], op=ALU.mult)
                    nc.vector.tensor_tensor(
                        out=Ct[:, cs], in0=Ct[:, cs], in1=Mt[:, cs], op=ALU.add)
                nc.scalar.activation(out=TC[:], in_=Ct[:], func=ACTF.Tanh)
                for li in range(2):
                    o0, cs = li * 768, slice(li * 256, li * 256 + 256)
                    nc.vector.tensor_tensor(
                        out=Ht[:, cs], in0=SIG[:, o0 + 512:o0 + 768],
                        in1=TC[:, cs], op=ALU.mult)
                nc.sync.dma_start(out=Xe_out[t], in_=Ht[:])
                hprev = Ht

    nc.compile()
    return nc


def _prep(Wih, Whh, bih, bhh, kin):
    """Permute gate blocks i,f,g,o -> i,f,o,g; return (WihT, WhhT, bias)."""
    perm = np.concatenate([np.arange(0, 256), np.arange(384, 512),
                           np.arange(256, 384)])
    Wihp = np.asarray(Wih)[perm].astype(np.float32)       # (512, kin)
    Whhp = np.asarray(Whh)[perm].astype(np.float32)       # (512, 128)
    bp = (np.asarray(bih) + np.asarray(bhh))[perm].astype(np.float32)
    return Wihp, Whhp, bp


def kernel(X, y_prev, Wa, ba, Wih1, Whh1, bih1, bhh1, Wih2, Whh2, bih2, bhh2):
    X = np.asarray(X, np.float32)
    y_prev = np.asarray(y_prev, np.float32)
    waf = np.asarray(Wa, np.float32)[2 * H:, 0]           # (127,)

    Wih1p, Whh1p, b1 = _prep(Wih1, Whh1, bih1, bhh1, 16)
    Wih2p, Whh2p, b2 = _prep(Wih2, Whh2, bih2, bhh2, 15)

    WhhT1 = np.ascontiguousarray(Whh1p.T).astype(BF16)    # (128, 512)
    WhhT2 = np.ascontiguousarray(Whh2p.T).astype(BF16)
    WihT1 = np.ascontiguousarray(Wih1p.T)                 # (16, 512)
    WihT2 = np.zeros((16, 512), np.float32)
    WihT2[0:15] = Wih2p.T
    WihT2[15] = b2                                        # bias via const lane

    def stack_wih(WihT):                                  # -> (128, 8, 512)
        s = np.zeros((128, 8, 512), np.float32)
        for rr in range(8):
            s[16 * rr:16 * rr + 16, rr, :] = WihT
        return s.reshape(128, 8 * 512).astype(BF16)

    WihS1 = stack_wih(WihT1)
    WihS2 = stack_wih(WihT2)
    wafrep = np.broadcast_to(waf, (128, T)).copy()

    l1_bias = bool(np.any(b1 != 0.0))
    key = l1_bias
    if key not in _cache:
        _cache[key] = _build(l1_bias)
    nc = _cache[key]

    in_maps = []
    for c in range(NCORES):
        bsl = slice(c * BC, (c + 1) * BC)
        m = {
            "Xs": np.ascontiguousarray(X[bsl].reshape(BC, T * 15)),
            "ys": np.ascontiguousarray(y_prev[bsl]),
            "wafrep": wafrep,
            "WhhT1": WhhT1, "WhhT2": WhhT2,
            "WihS1": WihS1, "WihS2": WihS2,
        }
        if l1_bias:
            b1p = np.zeros((128, 4), np.float32)
            for q in range(4):
                b1p[:, q] = b1[q * 128:(q + 1) * 128]
            m["b1p"] = b1p
        in_maps.append(m)

    global _last_in_maps
    _last_in_maps = in_maps
    res = run_bass_kernel_spmd(nc, in_maps, list(range(NCORES)))

    X_tilde = np.empty((B, T, K), np.float32)
    X_encoded = np.empty((B, T, 2 * H), np.float32)
    for c in range(NCORES):
        bsl = slice(c * BC, (c + 1) * BC)
        xt = np.asarray(res.results[c]["Xt_out"])         # (2, 128, 2032)
        X_tilde[bsl] = xt.reshape(BC, T, K)
        xe = np.asarray(res.results[c]["Xe_out"]).astype(np.float32)
        # (T, 128, 512) -> [t, h, l*256+b] -> out[b, t, l*128+h]
        v = xe.reshape(T, 128, 2, 256).transpose(3, 0, 2, 1).reshape(BC, T, 2 * H)
        X_encoded[bsl] = v
    return X_tilde, X_encoded
